# revision 15
# baseline (speedup 1.0000x reference)
# Trainium2 Bass kernel for nn_Attention_88313117540497.
#
# Reference computation (per batch b of 128):
#   v = x_b @ Wv                      (196, 384) @ (384, 512)
#   conv: each of the 512 channels' 14x14 image convolved with a 27x27
#         kernel qk at padding 13 -> same 14x14 output
#   y = conv_out @ Wo + bo            (196, 512) @ (512, 384)
#
# Math restructuring:
#  1. The 27x27 kernel at padding 13 covers every input pixel for every
#     output pixel, so the conv is a dense 196x196 linear map M over
#     positions, shared by all batches/channels: conv == matmul.
#  2. Folding W = Wv @ Wo (384x384) removes INNER=512:
#     y_b = (M @ x_b) @ W + bo.
#  3. All-transposed, M-first dataflow minimizes PE streaming cycles:
#       Z.T[d,p] = sum_u X[u,d] MT[u,p]   (lhsT = X chunk, rhs = MT)
#       Y.T[e,p] = sum_d W[d,e] Z.T[d,p]  (lhsT = W tile,  rhs = Z.T)
#     2940 PE cycles/batch vs 3840 for the W-first token-major order.
#  4. bf16 everywhere (4.4e-3 max-normalized error vs the 2e-2 budget):
#     1 cycle/row at any N, single fast LDWEIGHTS per matmul, half the
#     HBM bytes both directions. fp8 is DEAD for this problem: even ONE
#     operand in e4m3 (W-only or Z-only) measures 2.7-2.8e-2 error.
#
# Schedule (from iterative trace analysis across two sessions):
#  - Measured program anatomy (trace ns; metric = span * 1.2/1.388):
#    0-7.06 walrus preamble + TileContext barrier (fixed), first data
#    matmul ~10.6-11.7 (gated by x/mt DMA: ring cold-start ~110-150GB/s
#    until ~400KB have moved, then ~275GB/s), data stream ~20.7 (PE
#    floor 5880 cols/group @2.4GHz), tail evict+store+receipt ~3.6,
#    final barrier ~0.5, framework epilogue ~8.1 (fixed: walrus clears
#    all 256 semaphores, Tensor's 52-clear chain at ~115ns/clear is the
#    critical path and is issue-rate-bound, NOT clock-throttle-bound -
#    A/B'd: PE dummy-matmul streaming through the tail doesn't help).
#  - x is host-packed feature-transposed; the 68-valid-row token tail
#    chunk is shipped DE-PADDED as a separate [68, nb*384] transfer
#    (rows 68:128 of the tail lhsT are left as SBUF garbage - safe
#    because mt's matching rhs rows are zeros), cutting early-ring
#    bytes 23%. All x + mt ride the sync HWDGE ring, small groups
#    first; mt is fused with batch 0's main chunk into one first
#    transfer (wide packets - standalone mt's 784B packets move at
#    ~70GB/s cold and gated first-data at ~11.4-11.5).
#  - w rides the scalar HWDGE ring alone (lands ~11.3, first YT needs
#    it ~13); bias on SWDGE. SWDGE is never used for bulk or final
#    stores (~2.4us Q7 descriptor-drain tail, re-measured).
#  - PE clock (HAM): the array idles at 1.2GHz (50% issue duty
#    chip-wide) and unthrottles only after ~3.5-4.5us of SUSTAINED
#    streaming; a >0.6us gap in the stream RESETS the sustain timer
#    (measured: a 0.78us warm->data gap pushed unthrottle to 15.9us and
#    cost +2.3us). Warm matmul count must bridge the preamble to the
#    first data matmul with no gap: 13 x N=256 warms end ~10.5 for the
#    ~10.6 depadded first-data gate.
#  - ZT/YT software-pipelined one group apart; stage-1 PSUM evictions
#    on scalar (ACT copy+cast), stage-2 on vector (tensor_scalar
#    bias-add+cast); PSUM: z tags 2+2+1 banks, y tags 1+1+1, warms
#    share z0 = 8 banks exactly.
#  - y stores: 2-group grains on sync/gpsimd mid-kernel; last two
#    groups per-group on the two HWDGE rings; the very last tile is
#    evicted/stored in halves on both engine/ring pairs. Tail floor is
#    receipt-bound (~1.5us trigger-to-receipt + ~0.65us per-ring
#    serialization); merged 3-tile stores and 6-way splits both
#    re-measured NO BETTER than this scheme.
#
# Sharding: data-parallel over batch, 16 batches per core, weights
# replicated. No collectives.

import numpy as np
import ml_dtypes

import concourse.bass as bass
from concourse import bacc
import concourse.mybir as mybir
import concourse.tile as tile
from concourse.bass_utils import run_bass_kernel_spmd

N_CORES = 8
B = 128                 # total batch
BPC = B // N_CORES      # batches per core
DIM = 384
NPOS = 196              # 14*14 positions
IMG = 14
KS = 27                 # conv kernel size
U1 = NPOS - 128         # valid rows in token chunk 1 (68)

F32 = mybir.dt.float32
BF16 = mybir.dt.bfloat16
NP_BF16 = ml_dtypes.bfloat16

NG = BPC // 2           # 2-batch compute groups
GW = 2 * NPOS           # output cols per group: 392
# x load groups (start batch, count) on the sync HWDGE ring. Batch 0's
# main chunk is fused with mt; each group ships its main [128, nb*384]
# chunk then its depadded [68, nb*384] tail chunk.
XG = [(1, 1), (2, 2), (4, 2), (6, 2), (8, 4), (12, 4)]
NXT = 2 * len(XG) + 2   # total x tiles (c0+c1 per group, +mtx0c0+x0c1)


def build_program():
    nc = bacc.Bacc("TRN2", debug=False)

    # main chunks: xp0[p, b*384 + d] = x[b, p, d] for p in 0:128
    xp0_d = nc.dram_tensor("xp0", [128, BPC * DIM], BF16,
                           kind="ExternalInput")
    # token-tail chunks: xp1[p, b*384 + d] = x[b, 128+p, d] for p in 0:68
    xp1_d = nc.dram_tensor("xp1", [U1, BPC * DIM], BF16,
                           kind="ExternalInput")
    # fused first transfer: mt (cols 0:392) + batch 0 main chunk
    # (cols 392:776). mt packed: cols 0:196 = MT[0:128,:]; cols 196:392
    # = MT[128:196,:] on rows 0:68, ZEROS on rows 68:128 (these zeros
    # are what make the garbage rows of the depadded tail lhsT safe).
    mtx_d = nc.dram_tensor("mtx0", [128, GW + DIM], BF16,
                           kind="ExternalInput")
    # W folded, tiled: block k*3+j = W[128k:128k+128, 128j:128j+128]
    w_d = nc.dram_tensor("w", [128, 9 * 128], BF16, kind="ExternalInput")
    bias_d = nc.dram_tensor("bias", [128, 3], F32, kind="ExternalInput")
    # y transposed: [e-chunk, e%128, batch-token stream]
    y_d = nc.dram_tensor("y", [3, 128, BPC * NPOS], BF16,
                         kind="ExternalOutput")

    with tile.TileContext(nc) as tc:
        with (
            tc.tile_pool(name="const", bufs=1) as const,
            tc.tile_pool(name="work", bufs=2) as work,
            tc.tile_pool(name="psum", bufs=2, space="PSUM") as psum,
        ):
            # ---- PE warm-up feeder ----
            warm_sb = const.tile([128, 256], BF16)
            nc.gpsimd.memset(warm_sb[:, :], 1.0)

            # ---- loads: mt + x0 main fused, first on sync; x0 tail
            # second; then per-group main+tail pairs, all on sync.
            # w alone on scalar; bias on SWDGE. ----
            mtx_sb = const.tile([128, GW + DIM], BF16)
            nc.sync.dma_start(mtx_sb[:, :], mtx_d[:, :])
            mt_sb = mtx_sb  # rhs views mtx_sb[:, 0:GW]
            x0c1 = work.tile([128, DIM], BF16, tag="xp", bufs=NXT,
                             name="x0c1")
            nc.sync.dma_start(x0c1[0:U1, 0:DIM], xp1_d[:, 0:DIM])
            # xp_t[b] = (main tile, main off, tail tile, tail off)
            xp_t = {0: (mtx_sb, GW, x0c1, 0)}
            w_sb = const.tile([128, 9 * 128], BF16)
            nc.scalar.dma_start(w_sb[:, :], w_d[:, :])
            bias_sb = const.tile([128, 3], F32)
            nc.gpsimd.dma_start(bias_sb[:, :], bias_d[:, :])

            for gi, (s, nb) in enumerate(XG):
                t0 = work.tile([128, nb * DIM], BF16, tag="xp", bufs=NXT,
                               name=f"xc0_{gi}")
                nc.sync.dma_start(t0[:, 0:nb * DIM],
                                  xp0_d[:, s * DIM:(s + nb) * DIM])
                t1 = work.tile([128, nb * DIM], BF16, tag="xp", bufs=NXT,
                               name=f"xc1_{gi}")
                nc.sync.dma_start(t1[0:U1, 0:nb * DIM],
                                  xp1_d[:, s * DIM:(s + nb) * DIM])
                for b in range(s, s + nb):
                    xp_t[b] = (t0, (b - s) * DIM, t1, (b - s) * DIM)

            # ---- PE warm-up: WIDE (N=256) matmuls on the memset tile,
            # bridging from the preamble to the first data matmul with
            # no >0.6us gap (a gap resets the HAM sustain timer). ----
            for wi in range(13):
                warm = psum.tile([128, 256], F32, tag="z0", name=f"warm{wi}")
                nc.tensor.matmul(
                    warm[0:1, :], lhsT=warm_sb[:, 0:1], rhs=warm_sb[:, :],
                    start=True, stop=True,
                )

            # ---- main loop: ZT(g) emitted one group ahead of YT(g) ----
            ZBUFS = [2, 2, 1]
            zsb_g = {}

            def emit_zt(g):
                ba, bb = 2 * g, 2 * g + 1
                zps = [psum.tile([128, GW], F32, tag=f"z{k}",
                                 bufs=ZBUFS[k], name=f"zp{k}_{g}")
                       for k in range(3)]
                # batch-outer emission: all six of batch a's matmuls run
                # before batch b's first, so b's load may land ~0.5us
                # later without stalling the PE
                for half, b in ((0, ba), (1, bb)):
                    t0, o0, t1, o1 = xp_t[b]
                    c0 = half * NPOS
                    for k in range(3):
                        nc.tensor.matmul(
                            zps[k][:, c0:c0 + NPOS],
                            lhsT=t0[:, o0 + k * 128:o0 + (k + 1) * 128],
                            rhs=mt_sb[:, 0:NPOS],
                            start=True, stop=False,
                        )
                        # tail chunk: K=68 contraction - rows 68:128 of
                        # the depadded tile (SBUF garbage, possibly NaN
                        # bit patterns: 0*NaN=NaN!) never enter the PE
                        nc.tensor.matmul(
                            zps[k][:, c0:c0 + NPOS],
                            lhsT=t1[0:U1, o1 + k * 128:o1 + (k + 1) * 128],
                            rhs=mt_sb[0:U1, NPOS:GW],
                            start=False, stop=True,
                        )
                zsb = []
                for k in range(3):
                    z = work.tile([128, GW], BF16, tag=f"zsb{k}", bufs=2,
                                  name=f"zsb{k}_{g}")
                    # stage-1 evictions on scalar (ACT copy + cast)
                    nc.scalar.copy(z[:, :], zps[k][:, :])
                    zsb.append(z)
                zsb_g[g] = zsb

            ysb = {}

            def emit_yt(g):
                zsb = zsb_g.pop(g)
                pair, half = g // 2, g % 2
                last2 = g >= NG - 2
                for j in range(3):
                    yp = psum.tile([128, GW], F32, tag=f"y{j}", bufs=1,
                                   name=f"yp{j}_{g}")
                    for k in range(3):
                        nc.tensor.matmul(
                            yp[:, :],
                            lhsT=w_sb[:, (k * 3 + j) * 128:
                                      (k * 3 + j + 1) * 128],
                            rhs=zsb[k][:, :],
                            start=(k == 0), stop=(k == 2),
                        )
                    if last2:
                        yt = work.tile([128, GW], BF16, tag=f"ysb{j}", bufs=2,
                                       name=f"ysb{j}_{g}")
                        dst = yt[:, 0:GW]
                    else:
                        if half == 0:
                            ysb[j] = work.tile([128, 2 * GW], BF16,
                                               tag=f"ysb{j}", bufs=2,
                                               name=f"ysb{j}_{pair}")
                        yt = ysb[j]
                        dst = yt[:, half * GW:(half + 1) * GW]
                    # stage-2 evictions on vector (bias add + cast); for
                    # the final two groups spread j=1 to scalar so the
                    # last eviction chain (which gates the final store
                    # receipts) is two ops deep instead of three
                    final = g == NG - 1
                    if final and j == 2:
                        # very last tile: evict in halves on two engines
                        # in parallel, store halves on the two HWDGE
                        # rings - the tail is evict/2 + trigger + receipt
                        nc.vector.tensor_scalar_add(
                            dst[:, 0:NPOS], yp[:, 0:NPOS],
                            bias_sb[:, j:j + 1])
                        nc.scalar.activation(
                            dst[:, NPOS:GW], yp[:, NPOS:GW],
                            mybir.ActivationFunctionType.Identity,
                            bias=bias_sb[:, j:j + 1],
                        )
                        nc.sync.dma_start(
                            y_d[j, :, g * GW:g * GW + NPOS],
                            yt[:, 0:NPOS])
                        nc.scalar.dma_start(
                            y_d[j, :, g * GW + NPOS:(g + 1) * GW],
                            yt[:, NPOS:GW])
                        continue
                    if last2 and j == 1:
                        nc.scalar.activation(
                            dst, yp[:, :],
                            mybir.ActivationFunctionType.Identity,
                            bias=bias_sb[:, j:j + 1],
                        )
                    else:
                        nc.vector.tensor_scalar_add(dst, yp[:, :],
                                                    bias_sb[:, j:j + 1])
                    if last2:
                        # small final transfers, HWDGE rings only (a
                        # SWDGE store here leaves a ~2.4us Q7 tail)
                        eng = (nc.sync, nc.scalar, nc.sync)[j]
                        eng.dma_start(
                            y_d[j, :, g * GW:(g + 1) * GW], yt[:, 0:GW])
                    elif half == 1:
                        eng = (nc.sync, nc.sync, nc.gpsimd)[j]
                        eng.dma_start(
                            y_d[j, :, pair * 2 * GW:(pair + 1) * 2 * GW],
                            yt[:, 0:2 * GW])

            emit_zt(0)
            for g in range(NG):
                if g + 1 < NG:
                    emit_zt(g + 1)
                emit_yt(g)

    nc.compile()
    return nc


_PROGRAM = None


def _get_program():
    global _PROGRAM
    if _PROGRAM is None:
        _PROGRAM = build_program()
    return _PROGRAM


def _host_prep(x, Wv, qk, Wo, bo):
    x = np.asarray(x, dtype=np.float32)
    xc = x.reshape(N_CORES, BPC, NPOS, DIM)
    # main chunks [core, 128, b*384+d] and depadded tails [core, 68, ...]
    xp0 = np.ascontiguousarray(
        xc[:, :, 0:128, :].transpose(0, 2, 1, 3).reshape(
            N_CORES, 128, BPC * DIM)
    ).astype(NP_BF16)
    xp1 = np.ascontiguousarray(
        xc[:, :, 128:NPOS, :].transpose(0, 2, 1, 3).reshape(
            N_CORES, U1, BPC * DIM)
    ).astype(NP_BF16)
    # W = Wv @ Wo folded once, tiled [128, 9*128] with block k*3+j
    W = (np.asarray(Wv, np.float32) @ np.asarray(Wo, np.float32))
    wt = np.ascontiguousarray(
        W.reshape(3, 128, 3, 128).transpose(1, 0, 2, 3).reshape(128, 9 * 128)
    ).astype(NP_BF16)
    # MT[(u,v),(p,q)] = qk[13+u-p, 13+v-q]
    qk2 = np.asarray(qk, np.float32).reshape(KS, KS)
    idx = (KS // 2) + np.arange(IMG)[:, None] - np.arange(IMG)[None, :]
    MT = qk2[idx[:, None, :, None], idx[None, :, None, :]].reshape(NPOS, NPOS)
    mt = np.zeros((128, GW), np.float32)
    mt[:, 0:NPOS] = MT[0:128, :]
    mt[0:U1, NPOS:GW] = MT[128:NPOS, :]
    mt = mt.astype(NP_BF16)
    # fused first transfer: mt + batch 0's main chunk, per core
    mtx0 = np.ascontiguousarray(np.concatenate(
        [np.broadcast_to(mt, (N_CORES, 128, GW)), xp0[:, :, 0:DIM]], axis=2))
    bias = np.zeros((128, 3), np.float32)
    bias[:, 0] = np.asarray(bo, np.float32)[0:128]
    bias[:, 1] = np.asarray(bo, np.float32)[128:256]
    bias[:, 2] = np.asarray(bo, np.float32)[256:384]
    return xp0, xp1, mtx0, wt, bias


def _unpack_core(y2):
    # y2: [3, 128, BPC*NPOS] bf16 -> (BPC, NPOS, DIM) fp32
    return np.ascontiguousarray(
        np.asarray(y2).reshape(3, 128, BPC, NPOS).transpose(2, 3, 0, 1)
        .reshape(BPC, NPOS, DIM)
    ).astype(np.float32)


def _run(x, Wv, qk, Wo, bo, **spmd_kwargs):
    xp0, xp1, mtx0, wt, bias = _host_prep(x, Wv, qk, Wo, bo)
    nc = _get_program()
    in_maps = [
        {"xp0": xp0[c], "xp1": xp1[c], "mtx0": mtx0[c], "w": wt,
         "bias": bias}
        for c in range(N_CORES)
    ]
    res = run_bass_kernel_spmd(nc, in_maps, list(range(N_CORES)), **spmd_kwargs)
    y = np.concatenate(
        [_unpack_core(res.results[c]["y"]) for c in range(N_CORES)], axis=0)
    return y, res


def kernel(x, Wv, qk, Wo, bo):
    y, _ = _run(x, Wv, qk, Wo, bo)
    return y
